# revision 37
# baseline (speedup 1.0000x reference)
"""Multi-head self-attention Trainium2 kernel v3 (8 NeuronCores, SPMD).

Sharding: data-parallel over batch B=8 -> one batch element per core.

Single-core pipeline (bf16 matmuls, fp32 PSUM):
  qkvT = (x @ w_qkv)^T            q,k transposed; v natural+augmented
  sT_h[m,n] = k_h @ q_h^T         keys on partitions, queries free
  expT = exp(sT)                  scores in ~[-2,2]: no max subtraction
  outT_h = [v_h | 1]^T @ expT     ones column gives softmax denominator
  out_h = outT_h[:64] / outT_h[64]
  yT = w_proj^T @ outT + b_proj

v3 vs v2: row-group ping-pong for the tensor engine.
  Every 128-contraction matmul is split into two 64-row halves.  The
  top halves accumulate in one PSUM bank at tile_position (0,0), the
  bottom halves in a second bank at (64,0) (walrus requires each
  accumulation group to keep a single tile position equal to the
  stationary tensor's base partition), and a fused DVE op merges the
  two partial banks (+bias) at drain.  Emitted as strict T,B,T,B
  streams, the two members of each slot run CONCURRENTLY on the two
  64-row halves of the PE array (per-subarray row tiling), and every
  LDWEIGHTS hides under the opposite half's in-flight matmul - the
  ~107ns exposed weight-load per full-array matmul of v2 disappears
  and the scores hl0/hl1 pairs genuinely overlap.
  - accumulation brackets (start/stop) are derived from per-group
    emission counters, so slot order is free
  - q/k projection chunks 0-3 move into the DMA-gated prologue;
    chunks 4-7 stream through the supersteps (lag 2)
  - v-projection woven into superstep c=0 so ACT's exp stream (131us)
    fits inside the superstep span instead of starting 40us late
  - ~4us of warm-up matmuls on a zeroed scratch tile hold the PE HAM
    clock-gate at 8/8 while the first input DMAs land
  - proj chunk 0 accumulates inside the AV(7) tail (k-chunk 7 last)
  - PSUM: psS 2x[128,1024] scores/proj/warm + psQ 2x[128,512] qk/vproj
    + psPO 2x[65|128,512] AV/vproj = 8 banks
"""

from contextlib import ExitStack

import numpy as np
import ml_dtypes

import concourse.bass as bass
import concourse.mybir as mybir
import concourse.tile as tile
from concourse import bacc

BF16 = mybir.dt.bfloat16
F32 = mybir.dt.float32
P = 128  # SBUF partitions
HP = 64  # half-partition (row-group granularity)


class Grp:
    """PSUM accumulation-group bracket: start on the first emitted half,
    stop on the n-th.  Makes bracket placement order-independent."""

    def __init__(self, n):
        self.n = n
        self.i = 0

    def flags(self):
        s, e = self.i == 0, self.i == self.n - 1
        self.i += 1
        assert self.i <= self.n
        return s, e


def build_module(N=1024, D=1024, H=16, DK=64, reps=1, warmup=20):
    KC = D // P           # feature chunks (8)
    MC = N // P           # token chunks (8)
    FREE = 512            # moving free-dim per matmul (one PSUM bank fp32)
    NF = N // FREE        # 2
    assert H == 2 * KC and DK == HP

    nc = bacc.Bacc("TRN2", target_bir_lowering=False, debug=False)

    xT_d = nc.dram_tensor("xT", [D, N], BF16, kind="ExternalInput").ap()
    wq_d = nc.dram_tensor("wq", [D, D], BF16, kind="ExternalInput").ap()
    wk_d = nc.dram_tensor("wk", [D, D], BF16, kind="ExternalInput").ap()
    wv_d = nc.dram_tensor("wv", [D, D], BF16, kind="ExternalInput").ap()
    wp_d = nc.dram_tensor("wp", [D, D], BF16, kind="ExternalInput").ap()
    bq_d = nc.dram_tensor("bq", [P, KC], F32, kind="ExternalInput").ap()
    bk_d = nc.dram_tensor("bk", [P, KC], F32, kind="ExternalInput").ap()
    bvb_d = nc.dram_tensor("bvb", [P, D], BF16, kind="ExternalInput").ap()
    bp_d = nc.dram_tensor("bp", [P, KC], F32, kind="ExternalInput").ap()
    yT_d = nc.dram_tensor("yT", [D, N], F32, kind="ExternalOutput").ap()

    xT_v = xT_d.rearrange("(c p) n -> p c n", p=P)
    wq_v = wq_d.rearrange("(c p) n -> p c n", p=P)
    wk_v = wk_d.rearrange("(c p) n -> p c n", p=P)
    wv_v = wv_d.rearrange("(c p) n -> p c n", p=P)
    wp_v = wp_d.rearrange("(c p) n -> p c n", p=P)
    yT_v = yT_d.rearrange("(c p) n -> p c n", p=P)

    with tile.TileContext(nc) as tc, ExitStack() as ctx:
        consts = ctx.enter_context(tc.tile_pool(name="consts", bufs=1))
        perst = ctx.enter_context(tc.tile_pool(name="perst", bufs=1))
        psS = ctx.enter_context(tc.tile_pool(name="psS", bufs=2, space="PSUM"))
        psQ = ctx.enter_context(tc.tile_pool(name="psQ", bufs=2, space="PSUM"))
        psPO = ctx.enter_context(tc.tile_pool(name="psPO", bufs=2, space="PSUM"))
        qp = ctx.enter_context(tc.tile_pool(name="qp", bufs=4))
        kp = ctx.enter_context(tc.tile_pool(name="kp", bufs=4))
        exA_p = ctx.enter_context(tc.tile_pool(name="exA", bufs=14))
        exB_p = ctx.enter_context(tc.tile_pool(name="exB", bufs=14))
        misc_p = ctx.enter_context(tc.tile_pool(name="misc", bufs=2))
        mrg_p = ctx.enter_context(tc.tile_pool(name="mrg", bufs=2))
        yst_p = ctx.enter_context(tc.tile_pool(name="ystp", bufs=2))

        wq_sb = consts.tile([P, KC, D], BF16)
        wk_sb = consts.tile([P, KC, D], BF16)
        wv_sb = consts.tile([P, KC, D], BF16)
        wp_sb = consts.tile([P, KC, D], BF16)
        bq_sb = consts.tile([P, KC], F32)
        bk_sb = consts.tile([P, KC], F32)
        bp_sb = consts.tile([P, KC], F32)
        bvb_sb = consts.tile([P, D], BF16)
        warm_sb = consts.tile([P, FREE], BF16)

        xT_sb = perst.tile([P, KC, N], BF16)
        vaug_sb = perst.tile([P, MC, H, DK + 1], BF16)
        oT_sb = perst.tile([P, KC, N], BF16)
        nc.vector.memset(vaug_sb[:, :, :, DK : DK + 1], 1.0)
        nc.vector.memset(warm_sb, 0.0)

        ROWS = (slice(0, HP), slice(HP, P))
        TPOS = ((0, 0), (HP, 0))

        def half(out, lhsT_fn, rhs_fn, r, grp):
            """One 64-row half-matmul at row-group r (0=top)."""
            start, stop = grp.flags()
            nc.tensor.matmul(
                out,
                lhsT=lhsT_fn(ROWS[r]),
                rhs=rhs_fn(ROWS[r]),
                start=start,
                stop=stop,
                tile_position=TPOS[r],
                skip_group_check=True,
            )

        def pair_tile_slots(pX, pY, lhs_fn, rhs_fn, nk=KC):
            """One logical output accumulated over nk contraction chunks:
            top-half group in bank pX, bottom-half group in bank pY.
            Yields nk slot closures; each runs (T_k || B_k)."""
            gX, gY = Grp(nk), Grp(nk)
            for k in range(nk):
                def slot(k=k):
                    half(pX, lhs_fn(k), rhs_fn(k), 0, gX)
                    half(pY, lhs_fn(k), rhs_fn(k), 1, gY)

                yield slot

        def run(slots):
            for s in slots:
                s()

        for _rep in range(reps):
            # ---- input DMA (weights once; x re-loaded per rep) ----
            # ordered by first use: (x,wq,wk) per chunk for the prologue,
            # then wv (vproj in superstep c=0), then the late tensors
            for c in range(KC):
                nc.sync.dma_start(out=xT_sb[:, c, :], in_=xT_v[:, c, :])
                if _rep == 0:
                    nc.sync.dma_start(out=wq_sb[:, c, :], in_=wq_v[:, c, :])
                    nc.sync.dma_start(out=wk_sb[:, c, :], in_=wk_v[:, c, :])
            if _rep == 0:
                nc.sync.dma_start(out=bq_sb, in_=bq_d)
                nc.sync.dma_start(out=bk_sb, in_=bk_d)
                for c in range(KC):
                    nc.sync.dma_start(out=wv_sb[:, c, :], in_=wv_v[:, c, :])
                nc.sync.dma_start(out=bvb_sb, in_=bvb_d)
                for c in range(KC):
                    nc.sync.dma_start(out=wp_sb[:, c, :], in_=wp_v[:, c, :])
                nc.sync.dma_start(out=bp_sb, in_=bp_d)

            # ---- HAM warm-up: dummy matmuls on zeroed scratch while the
            # first x/wq/wk chunks stream in (rep 0 only) ----
            if _rep == 0:
                for _w in range(warmup):
                    ps = psS.tile([P, N], F32, tag="psS", name="warm")
                    nc.tensor.matmul(
                        ps[:, 0:FREE],
                        lhsT=warm_sb[:, 0:P],
                        rhs=warm_sb,
                        start=True,
                        stop=True,
                        skip_group_check=True,
                    )

            # ---- q/k projection: one logical tile = (dst, chunk c, f) ----
            qt, kt = {}, {}

            def qk_tile(c, f, w_sb, b_sb, dst, pool, tag):
                pX = pool.tile([P, FREE], F32, tag=tag, name="qkX")
                pY = pool.tile([P, FREE], F32, tag=tag, name="qkY")
                fs = slice(f * FREE, (f + 1) * FREE)
                slots = list(
                    pair_tile_slots(
                        pX,
                        pY,
                        lambda k: lambda rows: w_sb[rows, k, c * P : (c + 1) * P],
                        lambda k: lambda rows: xT_sb[rows, k, fs],
                    )
                )

                def drain():
                    # walrus: a DVE op may read only ONE input from PSUM,
                    # so stage bank Y (+bias) through SBUF, then add bank X
                    tmpS = mrg_p.tile([P, FREE], BF16, tag="mrg", name="mrg")
                    nc.vector.tensor_scalar_add(
                        out=tmpS, in0=pY, scalar1=b_sb[:, c : c + 1]
                    )
                    nc.vector.tensor_add(out=dst[:, fs], in0=pX, in1=tmpS)

                return slots, drain

            def new_qkt(c):
                qt[c] = qp.tile([P, N], BF16, tag="q", name="qt")
                kt[c] = kp.tile([P, N], BF16, tag="k", name="kt")

            # prologue: chunks 0..3; the (f0,f1) tile pair alternates
            # between (psQ,psPO) and (psPO,psQ) per substage so bank reuse
            # is 16 slots away from the previous drain (no WAR stall)
            def emit_prologue_chunk(c):
                new_qkt(c)
                for i, (w_sb, b_sb, dst) in enumerate(
                    ((wq_sb, bq_sb, qt[c]), (wk_sb, bk_sb, kt[c]))
                ):
                    pools = (
                        [(psQ, "psQ"), (psPO, "psPO")]
                        if (2 * c + i) % 2 == 0
                        else [(psPO, "psPO"), (psQ, "psQ")]
                    )
                    s0, d0 = qk_tile(c, 0, w_sb, b_sb, dst, *pools[0])
                    s1, d1 = qk_tile(c, 1, w_sb, b_sb, dst, *pools[1])
                    for a, b in zip(s0, s1):
                        a()
                        b()
                    d0()
                    d1()

            def prologue_stream(c):
                """Lazy item stream of emit_prologue_chunk (tile allocation
                happens as items are consumed, keeping pool-ring WAR order
                aligned with emission)."""
                new_qkt(c)
                for i, (w_sb, b_sb, dst) in enumerate(
                    ((wq_sb, bq_sb, qt[c]), (wk_sb, bk_sb, kt[c]))
                ):
                    pools = (
                        [(psQ, "psQ"), (psPO, "psPO")]
                        if (2 * c + i) % 2 == 0
                        else [(psPO, "psPO"), (psQ, "psQ")]
                    )
                    s0, d0 = qk_tile(c, 0, w_sb, b_sb, dst, *pools[0])
                    s1, d1 = qk_tile(c, 1, w_sb, b_sb, dst, *pools[1])
                    for a, b in zip(s0, s1):
                        yield a
                        yield b
                    yield d0
                    yield d1

            # superstep stream: chunk c split over per-step slot lists; one
            # logical (dst, f) tile spans 8/nsteps steps (psQ banks)
            def qk_stream(c, nsteps=8):
                new_qkt(c)
                per = 32 // nsteps  # slots per step
                specs = [
                    (w, b, d, f)
                    for w, b, d in ((wq_sb, bq_sb, qt[c]), (wk_sb, bk_sb, kt[c]))
                    for f in range(NF)
                ]
                for w_sb, b_sb, dst, f in specs:
                    slots, drain = qk_tile(c, f, w_sb, b_sb, dst, psQ, "psQ")
                    for lo in range(0, 8, per):
                        yield slots[lo : lo + per], (
                            drain if lo + per >= 8 else None
                        )

            # ---- v projection: logical tile (m, f); f0 on psQ banks,
            # f1 on psPO (both free during superstep c=0) ----
            def vproj_tile(m, f):
                # pool parity alternates per stage: bank reuse lands a full
                # stage after the previous drain instead of adjacent to it
                pool, tag = ((psQ, "psQ"), (psPO, "psPO"))[(m + f) % 2]
                pX = pool.tile([P, FREE], F32, tag=tag, name="vX")
                pY = pool.tile([P, FREE], F32, tag=tag, name="vY")
                fs = slice(f * FREE, (f + 1) * FREE)
                slots = list(
                    pair_tile_slots(
                        pX,
                        pY,
                        lambda k: lambda rows: xT_sb[rows, k, m * P : (m + 1) * P],
                        lambda k: lambda rows: wv_sb[rows, k, fs],
                    )
                )
                vsl = vaug_sb[:, m, 8 * f : 8 * (f + 1), 0:DK]

                def drain():
                    tmpS = mrg_p.tile([P, FREE], BF16, tag="mrg", name="mrg")
                    nc.vector.tensor_add(out=tmpS, in0=pY, in1=bvb_sb[:, fs])
                    nc.vector.tensor_add(out=vsl, in0=pX, in1=tmpS)

                return slots, drain

            # ---- scores + exp for one (c, j): 2 paired slots ----
            ex_tiles = {}

            def scores_step(c, j):
                s0 = psS.tile([P, N], F32, tag="psS", name="s0")
                s1 = psS.tile([P, N], F32, tag="psS", name="s1")
                ex0 = exA_p.tile([P, N], BF16, tag="ex", name="ex")
                ex1 = exB_p.tile([P, N], BF16, tag="ex", name="ex")
                ex_tiles[(c, j, 0)] = ex0
                ex_tiles[(c, j, 1)] = ex1

                def mk(f):
                    def slot():
                        for r, s in ((0, s0), (1, s1)):
                            nc.tensor.matmul(
                                s[:, f * FREE : (f + 1) * FREE],
                                lhsT=kt[c][ROWS[r], j * P : (j + 1) * P],
                                rhs=qt[c][ROWS[r], f * FREE : (f + 1) * FREE],
                                start=True,
                                stop=True,
                                tile_position=TPOS[r],
                                skip_group_check=True,
                            )

                    return slot

                def mkexp(s, ex):
                    def runx():
                        nc.scalar.activation(
                            out=ex, in_=s, func=mybir.ActivationFunctionType.Exp
                        )

                    return runx

                return [mk(0), mk(1)], mkexp(s0, ex0), mkexp(s1, ex1)

            # ---- AV: logical tile (c, hl, f) = [65,512] x 2 psPO banks;
            # 8 slots over jj; drain merges, normalizes, writes oT ----
            # ---- AV: paired row-halves like the GEMMs — key contraction
            # split T/B into two psPO banks, merged through SBUF at drain
            # (tensor_copy then a single-PSUM-input add).  One logical
            # (hl, f) tile in flight; 4 paired slots per step. ----
            av_state = {}

            def av_step(c, j):
                """tl order (0,2,1,3): both f0 tiles finish by step 3 so
                the tail's proj weave sees oT chunk 7 f0 early."""
                tl = (0, 2, 1, 3)[j // 2]
                hl, f = tl // 2, tl % 2
                h = c * 2 + hl
                fs = slice(f * FREE, (f + 1) * FREE)
                if j % 2 == 0:
                    pX = psPO.tile([DK + 1, FREE], F32, tag="psPO", name="avX")
                    pY = psPO.tile([DK + 1, FREE], F32, tag="psPO", name="avY")
                    av_state[tl] = (
                        pX,
                        pY,
                        list(
                            pair_tile_slots(
                                pX,
                                pY,
                                lambda jj: lambda rows: vaug_sb[rows, jj, h, :],
                                lambda jj: lambda rows: ex_tiles[(c, jj, hl)][
                                    rows, fs
                                ],
                                nk=MC,
                            )
                        ),
                    )
                pX, pY, slots = av_state[tl]
                mms = slots[4 * (j % 2) : 4 * (j % 2) + 4]

                def post():
                    if j % 2 == 1:
                        drain_av(c, tl)

                return mms, post

            def drain_av(c, tl):
                """merge the half-banks through SBUF (one PSUM input per
                DVE op), then reciprocal -> broadcast -> normalize."""
                hl, f = tl // 2, tl % 2
                pX, pY, _ = av_state[tl]
                tmpS = misc_p.tile([DK + 1, FREE], BF16, tag="avs", name="avs")
                nc.vector.tensor_copy(out=tmpS, in_=pY)
                md = misc_p.tile([DK + 1, FREE], BF16, tag="avmd", name="avmd")
                nc.vector.tensor_add(out=md, in0=pX, in1=tmpS)
                rc = misc_p.tile([1, FREE], F32, tag="rc", name="rc")
                nc.vector.reciprocal(out=rc, in_=md[DK : DK + 1, :])
                rcb = misc_p.tile([DK, FREE], F32, tag="rcb", name="rcb")
                nc.gpsimd.partition_broadcast(rcb, rc)
                fs = slice(f * FREE, (f + 1) * FREE)
                if hl == 0:
                    nc.vector.tensor_mul(
                        out=oT_sb[0:DK, c, fs], in0=md[0:DK, :], in1=rcb
                    )
                else:
                    tmpo = misc_p.tile([DK, FREE], BF16, tag="tmpo", name="tmpo")
                    nc.vector.tensor_mul(out=tmpo, in0=md[0:DK, :], in1=rcb)
                    nc.sync.dma_start(out=oT_sb[DK:P, c, fs], in_=tmpo)

            # ---- output projection: logical tile (c, f) lives in the two
            # banks of one [128,1024] psS tile; k-chunk 7 deferrable ----
            def proj_tile(c, f):
                ps = psS.tile([P, N], F32, tag="psS", name="ps_proj")
                pX, pY = ps[:, 0:FREE], ps[:, FREE:N]
                fs = slice(f * FREE, (f + 1) * FREE)
                slots = list(
                    pair_tile_slots(
                        pX,
                        pY,
                        lambda k: lambda rows: wp_sb[rows, k, c * P : (c + 1) * P],
                        lambda k: lambda rows: oT_sb[rows, k, fs],
                    )
                )

                def drain():
                    tmpS = mrg_p.tile([P, FREE], BF16, tag="mrg", name="mrg")
                    nc.vector.tensor_scalar_add(
                        out=tmpS, in0=pY, scalar1=bp_sb[:, c : c + 1]
                    )
                    yst = yst_p.tile([P, FREE], F32, tag="yst", name="yst")
                    nc.vector.tensor_add(out=yst, in0=pX, in1=tmpS)
                    # last chunk drains on the idle ACT engine's queue so
                    # the final output DMAs parallel sync's queue backlog
                    eng = nc.scalar if c == KC - 1 else nc.sync
                    eng.dma_start(out=yT_v[:, c, fs], in_=yst)

                return slots, drain

            # ================= emission =================
            # prologue: qk chunks 0,1 plain (DMA-paced; warm-up dummies
            # cover it), then chunks 2,3 woven with scores(0) ONLY: each
            # weave step carries ~1.9us of qk work + 0.4us of scores, so
            # the step stays PE-bound while the ACT exp stream starts
            # ~16us earlier — slack that absorbs the thin ACT-bound stages
            # late in the superstep schedule.  (scores+exp self-pace at
            # ~2.05us/step through the 2-deep psS ring, so scores may only
            # be placed where co-scheduled PE work exceeds that.)
            emit_prologue_chunk(0)
            emit_prologue_chunk(1)

            def chain23():
                yield from prologue_stream(2)
                yield from prologue_stream(3)

            pitems = chain23()
            emitted = 0
            for j in range(MC):
                sc, exp0, exp1 = scores_step(0, j)
                sc[0]()
                target = (j + 1) * 72 // MC  # 72 items over 8 steps
                while emitted < target:
                    it = next(pitems, None)
                    if it is None:
                        break
                    it()
                    emitted += 1
                sc[1]()
                exp0()
                exp1()
            for it in pitems:
                it()

            # supersteps c=0..7: scores(c) for c>=1 + qk stream smoothed
            # over c=1..6 (chunks 4,5 half rate spanning two stages each,
            # 6,7 full rate) so every scores step carries enough PE work
            # to cover the ~2.05us/step ACT exp pace + vproj at c=0 +
            # AV(c-1) at c>=1
            qk_gen = None
            for c in range(KC):
                if c == 1:
                    qk_gen = qk_stream(4, 16)
                elif c == 3:
                    qk_gen = qk_stream(5, 16)
                elif c == 5:
                    qk_gen = qk_stream(6, 8)
                elif c == 6:
                    qk_gen = qk_stream(7, 8)
                for j in range(MC):
                    if c >= 1:
                        sc, exp0, exp1 = scores_step(c, j)
                    else:
                        sc = None
                    qks, qkpost = next(qk_gen, ([], None)) if c >= 1 else ([], None)
                    avmm, avpost = av_step(c - 1, j) if c >= 1 else ([], None)
                    # scores early (their exps feed next-stage AV), then qk
                    # pairs, then the AV matmuls as ONE burst at step end:
                    # a single full-array segment per step costs one LDW
                    # boundary instead of two, and the burst separates a qk
                    # tile's drain from its banks' reuse in the next step
                    if sc:
                        sc[0]()
                    for s in qks[0:2]:
                        s()
                    if sc:
                        sc[1]()
                        exp0()
                        exp1()
                    for s in qks[2:4]:
                        s()
                    for s in avmm:
                        s()
                    if qkpost:
                        qkpost()
                    if avpost:
                        avpost()
                    if c == 0:
                        vs0, vd0 = vproj_tile(j, 0)
                        vs1, vd1 = vproj_tile(j, 1)
                        for a, b in zip(vs0, vs1):
                            a()
                            b()
                        vd0()
                        vd1()

            # AV tail for chunk 7, woven with proj chunk 0 (k<7 slots only;
            # chunk-7 oT lands at tail steps 3 (f0) and 7 (f1))
            p00, d00 = proj_tile(0, 0)
            p01, d01 = proj_tile(0, 1)
            weave = [p00[k] for k in range(7)] + [p01[k] for k in range(7)]
            for j in range(MC):
                avmm, avpost = av_step(KC - 1, j)
                for m in avmm:
                    m()
                avpost()
                run(weave[2 * j : 2 * j + 2])
            run([p00[7], p01[7]])
            d00()
            d01()

            for c in range(1, KC):
                for f in range(NF):
                    slots, drain = proj_tile(c, f)
                    run(slots)
                    drain()

    nc.compile()
    return nc


def make_in_maps(x, w_qkv, b_qkv, w_proj, b_proj, N=1024, D=1024, H=16, DK=64):
    """Host-side prep: shard over batch, fold scale, transpose x, cast bf16."""
    bf = ml_dtypes.bfloat16
    KC = D // P
    scale = np.float32(1.0 / np.sqrt(DK))
    wq = np.ascontiguousarray((w_qkv[:, :D] * scale)).astype(bf)
    wk = np.ascontiguousarray(w_qkv[:, D : 2 * D]).astype(bf)
    wv = np.ascontiguousarray(w_qkv[:, 2 * D :]).astype(bf)
    wp = np.ascontiguousarray(w_proj).astype(bf)
    bq = np.ascontiguousarray((b_qkv[:D] * scale).reshape(KC, P).T).astype(np.float32)
    bk = np.ascontiguousarray(b_qkv[D : 2 * D].reshape(KC, P).T).astype(np.float32)
    bvb = np.ascontiguousarray(np.broadcast_to(b_qkv[2 * D :], (P, D))).astype(bf)
    bp = np.ascontiguousarray(b_proj.reshape(KC, P).T).astype(np.float32)
    in_maps = []
    for b in range(x.shape[0]):
        xT = np.ascontiguousarray(x[b].T).astype(bf)
        in_maps.append(
            dict(xT=xT, wq=wq, wk=wk, wv=wv, wp=wp, bq=bq, bk=bk, bvb=bvb, bp=bp)
        )
    return in_maps


_module_cache = {}


def kernel(x, w_qkv, b_qkv, w_proj, b_proj):
    from concourse.bass_utils import run_bass_kernel_spmd

    x = np.asarray(x)
    B = x.shape[0]
    if "nc" not in _module_cache:
        _module_cache["nc"] = build_module()
    nc = _module_cache["nc"]
    in_maps = make_in_maps(
        x, np.asarray(w_qkv), np.asarray(b_qkv), np.asarray(w_proj), np.asarray(b_proj)
    )
    res = run_bass_kernel_spmd(nc, in_maps, core_ids=list(range(B)))
    out = np.stack([np.asarray(r["yT"]).T for r in res.results], axis=0)
    return np.ascontiguousarray(out.astype(np.float32))
